# revision 15
# baseline (speedup 1.0000x reference)
"""Trainium2 Bass kernel for nn_BAKTTime: causal-conv frequency layer + LN + causal MHA.

Sharding: pure data-parallel over batch - 8 of the 64 batch items per NeuronCore,
no collectives. Each core runs the same 5-stage software-pipelined program over
its 8 batch items (S=512, D=512, H=8, DK=64).

v2: fp8(e4m3) DoubleRow matmuls where the error budget allows:
  - conv: 3-term error-compensated split (xh*Wh + xl*Wh + xh*Wl, one PSUM
    group; W pre-scaled x64 so the lo parts stay in fp8 normal range; LN is
    scale-invariant so the x64 never needs undoing). 24576 -> 18432 PE cyc/b.
  - q,k projections: single fp8 DoubleRow (h and Wq quantization errors are
    damped through softmax because score magnitudes are ~0.2); the x64*x64
    scale rides into the exp() activation scale (0.125/4096). 16384 -> 4096.
  - v / scores / ctx / out-projection stay bf16 (their quantization error
    would hit the output un-damped; DoubleRow needs fp8).

Per-batch dataflow:
  1. conv (token-major): a[s,o] = sum_{i,k} x[s+k-2,i] * W'[o,i,k] with the
     residual + sqrt_beta scaling folded into W' on host, so `a` IS emb+x.
     fp8 DR: x stored [128, {hi,lo}x{g}, {j}, S+4] with channel ch=g*256+j*128+p
     so each DR matmul contracts 256 channels; 18 DR matmuls per 256-wide
     output slice, all in one PSUM accumulation group.
  2. LN: bn_stats/bn_aggr per s-tile; rstd = exp(-0.5*ln(var+eps)) on ACT;
     h = (a-mean)*rstd fused into the PSUM->SBUF copy (ln_w folded into the
     QKV weights on host; ln_b/biases asserted zero).
  3. h -> hT via 16 HWDGE xbar-transpose DMAs (bf16 128x128 blocks) into
     channel-paired [128, 2, S] tiles; hh = fp8(hT) cast on Pool.
  4. Projections: qT,kT D-major [o,s] via fp8 DR; v token-major [s,o] bf16
     with a ones column per head (v_aug) so the ctx matmul also yields the
     softmax denominator row.
  5. Attention per head-PAIR (bf16): scoresT[k,q] for both heads in one
     [128, 2x512] PSUM tile (causal-trimmed); ONE exp (ACT, scale=2^-15,
     ->bf16) and ONE tri-mask multiply (DVE) per (pair,ki); ctxU[65,512]
     accumulates v_aug^T @ PT (row 64 = denominator; first-block column 0
     zeroed implements the reference zero_pad, +1e-10 guards q=0).
  6. Denominators: one Pool-SWDGE DMA gathers all 8 rows -> [8,512];
     reciprocal_approx_fast (DVE) -> bf16; per-head HWDGE broadcast
     [1->64,512] into one [128, 4, S] tile.
  7. Normalize: unnormalized ctx pairs live in one [128, 4, S] tile (odd
     heads moved up by a single partition-shifting DMA); one big DVE
     multiply by the broadcast reciprocals; output projection runs K=128
     bf16: 16 matmuls -> out[s,o] PSUM -> ACT copy -> one DRAM store.

The batch loop is a 5-deep software pipeline: each iteration's engine streams
carry [conv(b) | outproj(b-4) | normalize(b-3) | qkv+attention(b-1) |
denominator chain(b-2)], keeping PE busy; cross-engine chains hide under
neighboring batches' matmuls.
"""

import sys

sys.path.insert(0, "/opt/trn_rl_repo")

import numpy as np
import ml_dtypes
from contextlib import ExitStack

import concourse.bass as bass
from concourse import bacc
import concourse.mybir as mybir
import concourse.tile as tile
from concourse.bass_utils import run_bass_kernel_spmd

# Force Exp and Ln to resolve to the single table set that contains both
# (natural_log_exp_and_others), so ACT doesn't thrash table loads between
# exp_and_others and natural_log every batch (~2.7us per switch).
import concourse.hw_specs as _hw_specs

_orig_get_tables = _hw_specs.get_activation_tables


def _patched_get_tables(arch):
    t = dict(_orig_get_tables(arch))
    exp = mybir.ActivationFunctionType.Exp
    ln = mybir.ActivationFunctionType.Ln
    for name, funcs in t.items():
        if name != "natural_log_exp_and_others" and (exp in funcs or ln in funcs):
            t[name] = funcs - {exp, ln}
    return t


_hw_specs.get_activation_tables = _patched_get_tables
bacc.get_activation_tables = _patched_get_tables

import os

DEBUG_STAGE = os.environ.get("KDEBUG", "")

B, S, D, H, KW = 64, 512, 512, 8, 3
DK = D // H  # 64
NCORES = 8
BL = B // NCORES  # 8 batches per core
P = 128
NST = S // P  # 4 s-tiles
NIC = D // P  # 4 input-chunks
NG = 2  # channel pair-groups for DoubleRow (256 ch each)
SP = S + 16  # padded x free dim (2 zero cols + 512 + pad; pair stride must be 16B-aligned)
EPS = 1e-12
F32 = mybir.dt.float32
BF16 = mybir.dt.bfloat16
FP8 = mybir.dt.float8e4
AF = mybir.ActivationFunctionType
DR = mybir.MatmulPerfMode.DoubleRow
# q,k are computed from x64-scaled weights on both sides: scores carry 64*64;
# exp(s/8) becomes exp(s64 * 0.125/4096).
EXP_SCALE = 0.125 / 4096.0


def build_nc():
    nc = bacc.Bacc("TRN2", target_bir_lowering=False)
    # x: [b][p][hilo*2+g][j][s+2] = fp8 part of x[b, s, g*256+j*128+p]
    xt = nc.declare_dram_parameter("xt", [BL, P, 4 * NG * SP], FP8, isOutput=False)
    # wconv: [hilo*2+g][p][j][k][d] = fp8 part of 16*W'[d, g*256+j*128+p, k]
    wconv = nc.declare_dram_parameter("wconv", [4, P, NG, KW, D], FP8, isOutput=False)
    # wq/wk: [g][p][j][d] = fp8(64*Wq[d, g*256+j*128+p])
    wq = nc.declare_dram_parameter("wq", [NG, P, NG, D], FP8, isOutput=False)
    wk = nc.declare_dram_parameter("wk", [NG, P, NG, D], FP8, isOutput=False)
    wv = nc.declare_dram_parameter("wv", [NIC, P, D], BF16, isOutput=False)
    wo = nc.declare_dram_parameter("wo", [NIC, P, D], BF16, isOutput=False)
    trim = nc.declare_dram_parameter("trim", [P, 2, P], BF16, isOutput=False)
    out = nc.declare_dram_parameter("out", [BL, P, NST, D], F32, isOutput=True)

    with ExitStack() as ctx:
        tc = ctx.enter_context(tile.TileContext(nc))
        singles = ctx.enter_context(tc.tile_pool(name="singles", bufs=1))
        xt_pool = ctx.enter_context(tc.tile_pool(name="xt", bufs=2))
        a_pool = ctx.enter_context(tc.tile_pool(name="a", bufs=6))
        stat_pool = ctx.enter_context(tc.tile_pool(name="stat", bufs=4))
        h_pool = ctx.enter_context(tc.tile_pool(name="h", bufs=8))
        ht_pool = ctx.enter_context(tc.tile_pool(name="ht", bufs=2))
        hh_pool = ctx.enter_context(tc.tile_pool(name="hh", bufs=2))
        qk_pool = ctx.enter_context(tc.tile_pool(name="qk", bufs=16))
        v_pool = ctx.enter_context(tc.tile_pool(name="v", bufs=8))
        pt_pool = ctx.enter_context(tc.tile_pool(name="pt", bufs=6))
        dn_pool = ctx.enter_context(tc.tile_pool(name="dn", bufs=2))
        r_pool = ctx.enter_context(tc.tile_pool(name="r", bufs=2))
        cx_pool = ctx.enter_context(tc.tile_pool(name="cx", bufs=3))
        o_pool = ctx.enter_context(tc.tile_pool(name="o", bufs=3))
        ps_a = ctx.enter_context(tc.tile_pool(name="ps_a", bufs=2, space="PSUM"))
        ps_mm = ctx.enter_context(tc.tile_pool(name="ps_mm", bufs=2, space="PSUM"))
        ps_sc = ctx.enter_context(tc.tile_pool(name="ps_sc", bufs=1, space="PSUM"))
        ps_cx = ctx.enter_context(tc.tile_pool(name="ps_cx", bufs=2, space="PSUM"))

        # --- load weights once ---
        wconv_sb = [singles.tile([P, NG, KW, D], FP8, name=f"wconv{t}", tag=f"wconv{t}") for t in range(4)]
        wq_sb = [singles.tile([P, NG, D], FP8, name=f"wq{g}", tag=f"wq{g}") for g in range(NG)]
        wk_sb = [singles.tile([P, NG, D], FP8, name=f"wk{g}", tag=f"wk{g}") for g in range(NG)]
        wv_sb = [singles.tile([P, D], BF16, name=f"wv{i}", tag=f"wv{i}") for i in range(NIC)]
        wo_sb = [singles.tile([P, D], BF16, name=f"wo{i}", tag=f"wo{i}") for i in range(NIC)]
        trim_sb = singles.tile([P, 2, P], BF16, name="trim", tag="trim")
        eps_sb = singles.tile([P, 1], F32, name="eps", tag="eps")
        nc.vector.memset(eps_sb, EPS)
        tiny_sb = singles.tile([P, 1], F32, name="tiny", tag="tiny")
        nc.vector.memset(tiny_sb, 1e-10)
        zero_sb = singles.tile([P, 1], F32, name="zero", tag="zero")
        nc.vector.memset(zero_sb, 0.0)
        # conv weights + mask first (needed by iteration 0); projection
        # weights after (first needed one pipeline iteration later)
        for t in range(4):
            nc.gpsimd.dma_start(out=wconv_sb[t], in_=wconv[t])
        nc.gpsimd.dma_start(out=trim_sb, in_=trim[:])
        for g in range(NG):
            nc.gpsimd.dma_start(out=wq_sb[g], in_=wq[g])
            nc.gpsimd.dma_start(out=wk_sb[g], in_=wk[g])
        for i in range(NIC):
            nc.gpsimd.dma_start(out=wv_sb[i], in_=wv[i])
        for i in range(NIC):
            nc.gpsimd.dma_start(out=wo_sb[i], in_=wo[i])

        def tail_norm(b, csb, rec):
            # csb: [P, H//2, S] unnormalized ctx head-pairs; rec: [P, H//2, S]
            # broadcast reciprocals. One big multiply.
            csbn = cx_pool.tile([P, H // 2, S], BF16, name="csbn", tag="csbn")
            nc.vector.tensor_mul(csbn, csb, rec)
            if DEBUG_STAGE == "csbn":
                for hp in range(H // 2):
                    cf = a_pool.tile([P, S], F32, name="cf", tag="cf")
                    nc.scalar.copy(cf, csbn[:, hp, :])
                    nc.sync.dma_start(out=out[b, :, hp, :], in_=cf)
            return (b, csbn)

        def tail_mm(b, csbn):
            osb = o_pool.tile([P, NST, D], F32, name="osb", tag="osb")
            for st in range(NST):
                ops = ps_mm.tile([P, D], F32, name="qps", tag="qps")
                for hp in range(H // 2):
                    nc.tensor.matmul(
                        ops,
                        lhsT=csbn[:, hp, st * P : (st + 1) * P],
                        rhs=wo_sb[hp],
                        start=(hp == 0),
                        stop=(hp == H // 2 - 1),
                    )
                nc.scalar.copy(osb[:, st, :], ops)
            if not DEBUG_STAGE:
                nc.sync.dma_start(out=out[b], in_=osb)

        def load_xt(b):
            xsb = xt_pool.tile([P, 4, NG, SP], FP8, name="xsb", tag="xsb")
            nc.sync.dma_start(out=xsb.rearrange("p a j s -> p (a j s)"), in_=xt[b])
            return xsb

        def front(b, xsb):
            """conv + LN + h-transpose + fp8 cast for batch b."""
            mv = stat_pool.tile([P, NST, 2], F32, name="mv", tag="mv")
            lnv = stat_pool.tile([P, NST], F32, name="lnv", tag="lnv")
            rstd = stat_pool.tile([P, NST], F32, name="rstd", tag="rstd")
            ht_sb = [ht_pool.tile([P, NG, S], BF16, name=f"ht{g}", tag=f"ht{g}") for g in range(NG)]
            if DEBUG_STAGE == "x":
                for ii in range(4):
                    xf32 = a_pool.tile([P, D], F32, name="xf32", tag="xf32")
                    for jj in range(NG):
                        nc.scalar.copy(xf32[:, jj * 256 : (jj + 1) * 256], xsb[:, ii, jj, 2 : 2 + 256])
                    nc.sync.dma_start(out=out[b, :, ii, :], in_=xf32)
            a_list = []
            for st in range(NST):
                aps = ps_a.tile([P, D], F32, name="aps", tag="aps")
                for ds in range(2):
                    o_slc = aps[:, ds * 256 : (ds + 1) * 256]
                    n = 0
                    for xi, wi in ((0, 0), (1, 0), (0, 1)):
                        for g in range(NG):
                            for k in range(KW):
                                nc.tensor.matmul(
                                    o_slc,
                                    lhsT=xsb[:, xi * NG + g, :, st * P + k : st * P + k + P],
                                    rhs=wconv_sb[wi * NG + g][:, :, k, ds * 256 : (ds + 1) * 256],
                                    start=(n == 0),
                                    stop=(n == 3 * NG * KW - 1),
                                    perf_mode=DR,
                                )
                                n += 1
                asb = a_pool.tile([P, D], F32, name="asb", tag="asb")
                nc.vector.tensor_copy(asb, aps)
                if DEBUG_STAGE == "a":
                    nc.sync.dma_start(out=out[b, :, st, :], in_=asb)
                stats = stat_pool.tile([P, 6], F32, name="bnst", tag="bnst")
                nc.vector.bn_stats(out=stats, in_=asb)
                nc.vector.bn_aggr(out=mv[:, st, :], in_=stats)
                a_list.append(asb)
            nc.scalar.activation(lnv, mv[:, :, 1], AF.Ln, bias=eps_sb, scale=1.0)
            nc.scalar.activation(rstd, lnv, AF.Exp, bias=zero_sb, scale=-0.5)
            for st in range(NST):
                hsb = h_pool.tile([P, D], BF16, name="hsb", tag="hsb")
                nc.vector.tensor_scalar(
                    hsb,
                    a_list[st],
                    scalar1=mv[:, st, 0:1],
                    scalar2=rstd[:, st : st + 1],
                    op0=mybir.AluOpType.subtract,
                    op1=mybir.AluOpType.mult,
                )
                if DEBUG_STAGE == "h":
                    hf = a_pool.tile([P, D], F32, name="hf", tag="hf")
                    nc.scalar.copy(hf, hsb)
                    nc.sync.dma_start(out=out[b, :, st, :], in_=hf)
                for i in range(NIC):
                    nc.sync.dma_start(
                        out=ht_sb[i // 2][:, i % 2, st * P : (st + 1) * P],
                        in_=hsb[:, i * P : (i + 1) * P],
                        transpose=True,
                    )
            hh_sb = []
            for g in range(NG):
                hh = hh_pool.tile([P, NG, S], FP8, name=f"hh{g}", tag=f"hh{g}")
                nc.gpsimd.tensor_copy(hh, ht_sb[g])
                hh_sb.append(hh)
            return ht_sb, hh_sb

        def mid(b, ht_sb, hh_sb):
            """projections + attention for batch b. Returns tail state."""
            qt_sb = []
            kt_sb = []
            for oc in range(NIC):
                for dst, w_sb in ((qt_sb, wq_sb), (kt_sb, wk_sb)):
                    qps = ps_mm.tile([P, S], F32, name="qps", tag="qps")
                    for ss in range(2):
                        for g in range(NG):
                            nc.tensor.matmul(
                                qps[:, ss * 256 : (ss + 1) * 256],
                                lhsT=w_sb[g][:, :, oc * P : (oc + 1) * P],
                                rhs=hh_sb[g][:, :, ss * 256 : ss * 256 + 256],
                                start=(g == 0),
                                stop=(g == NG - 1),
                                perf_mode=DR,
                            )
                    qsb = qk_pool.tile([P, S], BF16, name="qtsb", tag="qtsb")
                    nc.vector.tensor_copy(qsb, qps)
                    if (DEBUG_STAGE == "q" and w_sb is wq_sb) or (DEBUG_STAGE == "k" and w_sb is wk_sb):
                        qf = a_pool.tile([P, S], F32, name="qf", tag="qf")
                        nc.vector.tensor_copy(qf, qps)
                        nc.sync.dma_start(out=out[b, :, oc, :], in_=qf)
                    dst.append(qsb)

            v_aug = []
            for st in range(NST):
                vps = ps_mm.tile([P, D], F32, name="qps", tag="qps")
                for i in range(NIC):
                    nc.tensor.matmul(
                        vps,
                        lhsT=ht_sb[i // 2][:, i % 2, st * P : (st + 1) * P],
                        rhs=wv_sb[i],
                        start=(i == 0),
                        stop=(i == NIC - 1),
                    )
                if DEBUG_STAGE == "v":
                    vf = a_pool.tile([P, D], F32, name="vf", tag="vf")
                    nc.vector.tensor_copy(vf, vps)
                    nc.sync.dma_start(out=out[b, :, st, :], in_=vf)
                vsb = v_pool.tile([P, H, 66], BF16, name="vsb", tag="vsb")
                nc.vector.memset(vsb[:, :, 64:66], 1.0)
                nc.vector.tensor_copy(
                    vsb[:, :, 0:64], vps.rearrange("p (h d) -> p h d", h=H)
                )
                v_aug.append(vsb)

            dtmp = dn_pool.tile([65, H, S], BF16, name="dtmp", tag="dtmp")
            csb = cx_pool.tile([P, H // 2, S], BF16, name="csb", tag="csb")
            codd = cx_pool.tile([DK, H // 2, S], BF16, name="codd", tag="codd")
            for hp in range(H // 2):
                cps2 = [
                    ps_cx.tile([65, S], F32, name="cps", tag="cps") for _ in range(2)
                ]
                for ki in range(NST):
                    qoff = ki * P
                    nq = S - qoff
                    sps = ps_sc.tile([P, 2, S], F32, name="sps", tag="sps")
                    for e in range(2):
                        hr = e * DK
                        nc.tensor.matmul(
                            sps[:, e, 0:nq],
                            lhsT=kt_sb[hp][hr : hr + DK, ki * P : (ki + 1) * P],
                            rhs=qt_sb[hp][hr : hr + DK, qoff:S],
                            start=True,
                            stop=True,
                        )
                    pt = pt_pool.tile([P, 2, S], BF16, name="pt", tag="pt")
                    nc.scalar.activation(
                        pt[:, :, 0:nq], sps[:, :, 0:nq], AF.Exp, scale=EXP_SCALE
                    )
                    tsl = trim_sb[:, 1 if ki == 0 else 0, :]
                    tbc = bass.AP(
                        tensor=tsl.tensor,
                        offset=tsl.offset,
                        ap=[tsl.ap[0], [0, 2], [1, P]],
                    )
                    nc.vector.tensor_mul(pt[:, :, 0:P], pt[:, :, 0:P], tbc)
                    for e in range(2):
                        nc.tensor.matmul(
                            cps2[e][:, qoff:S],
                            lhsT=v_aug[ki][:, 2 * hp + e, 0:65],
                            rhs=pt[:, e, 0:nq],
                            start=(ki == 0),
                            stop=(ki == NST - 1),
                        )
                # denominator rows -> staging (row 64), +tiny guard for q=0
                nc.scalar.activation(
                    dtmp[64:65, 2 * hp, :], cps2[0][64:65, :], AF.Identity, bias=tiny_sb[64:65, :], scale=1.0
                )
                nc.scalar.activation(
                    dtmp[64:65, 2 * hp + 1, :], cps2[1][64:65, :], AF.Identity, bias=tiny_sb[64:65, :], scale=1.0
                )
                # unnormalized ctx: even head -> rows 0-63 direct; odd head
                # staged for one partition-shifting DMA at the end.
                nc.scalar.copy(csb[0:DK, hp, :], cps2[0][0:DK, :])
                nc.scalar.copy(codd[:, hp, :], cps2[1][0:DK, :])

            nc.sync.dma_start(out=csb[DK:P, :, :], in_=codd)
            # issue the denominator gather now (DMA latency hides across the
            # pipeline); the reciprocal + broadcasts run one iteration later.
            dcat = dn_pool.tile([H, S], F32, name="dcat", tag="dcat")
            nc.gpsimd.dma_start(out=dcat, in_=dtmp[64:65, :, :])
            return (b, csb, dcat)

        def denom_chain(b, dcat):
            rcat = dn_pool.tile([H, S], F32, name="rcat", tag="rcat")
            nc.vector.reciprocal_approx_fast(out=rcat, in_=dcat)
            rcb = dn_pool.tile([H, S], BF16, name="rcb", tag="rcb")
            nc.vector.tensor_copy(rcb, rcat)
            rec = r_pool.tile([P, H // 2, S], BF16, name="rec", tag="rec")
            for hp in range(H // 2):
                for e in range(2):
                    rsrc = rcb[2 * hp + e : 2 * hp + e + 1, :]
                    rsrc = bass.AP(
                        tensor=rsrc.tensor,
                        offset=rsrc.offset,
                        ap=[rsrc.ap[0], [0, DK], [1, S]],
                    )
                    nc.sync.dma_start(out=rec[e * DK : (e + 1) * DK, hp, :], in_=rsrc)
            return rec

        if os.environ.get("KSTRIP") == "1":
            for b in range(BL):
                xsb = load_xt(b)
                front(b, xsb)
        elif os.environ.get("KSTRIP") == "2":
            for b in range(BL):
                xsb = load_xt(b)
                ht, hh = front(b, xsb)
                mid(b, ht, hh)
        elif os.environ.get("KSTRIP") == "3":
            for b in range(BL):
                xsb = load_xt(b)
                ht, hh = front(b, xsb)
                _, csb2, dcat2 = mid(b, ht, hh)
                rec2 = denom_chain(b, dcat2)
                tail_mm(*tail_norm(b, csb2, rec2))
        else:
            # 5-deep software pipeline over batches; per iteration the engine
            # streams carry [conv(b) | outproj(b-4) | normalize(b-3) |
            # qkv+attention(b-1) | denominator chain(b-2)].
            pend_mid = None
            pend_den = None
            pend_tail = None
            xt_cur = load_xt(0)
            pend_norm = None
            for b in range(BL):
                xt_next = load_xt(b + 1) if b + 1 < BL else None
                ht, hh = front(b, xt_cur)
                if pend_norm is not None:
                    tail_mm(*pend_norm)
                new_norm = tail_norm(*pend_tail) if pend_tail is not None else None
                new_den = mid(*pend_mid) if pend_mid is not None else None
                if pend_den is not None:
                    db, dcsb, ddcat = pend_den
                    new_tail = (db, dcsb, denom_chain(db, ddcat))
                else:
                    new_tail = None
                pend_mid = (b, ht, hh)
                pend_den = new_den
                pend_tail = new_tail
                pend_norm = new_norm
                xt_cur = xt_next
            # drain: collapse the remaining stages as tightly as dependencies allow
            if pend_norm is not None:
                tail_mm(*pend_norm)
            new_den = mid(*pend_mid)
            db, dcsb, ddcat = pend_den
            new_tail = (db, dcsb, denom_chain(db, ddcat))
            tail_mm(*tail_norm(*pend_tail))
            pend_den, pend_tail = new_den, new_tail
            db, dcsb, ddcat = pend_den
            new_tail = (db, dcsb, denom_chain(db, ddcat))
            tail_mm(*tail_norm(*pend_tail))
            tail_mm(*tail_norm(*new_tail))

    nc.compile()
    return nc


def prep_inputs(inputs):
    """Host-side prep: shard over batch, fold scales into weights, fp8 splits."""
    x = np.asarray(inputs["x"], np.float32)
    conv_w = np.asarray(inputs["conv_w"], np.float32)
    conv_b = np.asarray(inputs["conv_b"], np.float32)
    sb = np.asarray(inputs["sqrt_beta"], np.float32).reshape(D)
    ln_w = np.asarray(inputs["ln_w"], np.float32)
    ln_b = np.asarray(inputs["ln_b"], np.float32)
    Wq = np.asarray(inputs["Wq"], np.float32)
    Wk = np.asarray(inputs["Wk"], np.float32)
    Wv = np.asarray(inputs["Wv"], np.float32)
    Wo = np.asarray(inputs["Wo"], np.float32)
    mask = np.asarray(inputs["mask"])

    for nm in ("bq", "bk", "bv", "bo"):
        assert not np.any(np.asarray(inputs[nm])), f"{nm} must be zero"
    assert not np.any(conv_b), "conv_b must be zero"
    assert not np.any(ln_b), "ln_b must be zero"
    assert np.array_equal(
        mask.reshape(S, S), np.tril(np.ones((S, S), mask.dtype))
    ), "mask must be causal"

    bf = ml_dtypes.bfloat16
    f8 = ml_dtypes.float8_e4m3fn

    c1 = 1.0 - sb * sb
    c2 = 1.0 + sb * sb
    Wp = conv_w * c1[:, None, None]  # [o, i, k]
    Wp[np.arange(D), np.arange(D), 2] += c2
    Wp16 = Wp * 16.0  # x16 (not x64): diag taps c2 up to ~17 must stay under e4m3 max 448
    Wph = Wp16.astype(f8)
    Wpl = (Wp16 - Wph.astype(np.float32)).astype(f8)
    # wconv[hilo*2+g][p][j][k][d] = part[d, g*256+j*128+p, k]
    wconv = np.empty((4, P, NG, KW, D), f8)
    for t, Wpart in enumerate((Wph, Wpl)):
        # Wpart: [d, ch, k] -> [ch, k, d] -> [g, j, p, k, d] -> [g, p, j, k, d]
        r = np.ascontiguousarray(Wpart.transpose(1, 2, 0)).reshape(NG, NG, P, KW, D)
        wconv[2 * t : 2 * t + 2] = r.transpose(0, 2, 1, 3, 4)

    def fold_qk(W):  # [o, i] -> [g, p, j, o], fp8(64*W*ln_w)
        Wf = (64.0 * W * ln_w[None, :]).astype(f8)
        r = np.ascontiguousarray(Wf.T).reshape(NG, NG, P, D)
        return r.transpose(0, 2, 1, 3).copy()

    wq_h, wk_h = fold_qk(Wq), fold_qk(Wk)

    def fold(W):  # [o, i] -> [ic, il, o] with ln_w folded on i
        Wf = W * ln_w[None, :]
        return np.ascontiguousarray(Wf.T).reshape(NIC, P, D)

    wv_h = fold(Wv).astype(bf)
    wo_h = np.ascontiguousarray(Wo.T).reshape(NIC, P, D).astype(bf)

    tri = np.triu(np.ones((P, P), np.float32))
    tri0 = tri.copy()
    tri0[:, 0] = 0.0
    trim = np.stack([tri, tri0], axis=1)  # [P, 2, P]

    consts = {
        "wconv": wconv,
        "wq": wq_h,
        "wk": wk_h,
        "wv": wv_h,
        "wo": wo_h,
        "trim": trim.astype(bf),
    }

    in_maps = []
    for c in range(NCORES):
        xs = x[c * BL : (c + 1) * BL]  # [BL, S, D]
        xh = xs.astype(f8)
        xl = (xs - xh.astype(np.float32)).astype(f8)
        xtp = np.zeros((BL, P, 4, NG, SP), f8)
        for t, xpart in enumerate((xh, xl)):
            # xpart [b, s, ch] -> [b, ch, s] -> [b, g, j, p, s] -> [b, p, (t,g), j, s]
            r = np.ascontiguousarray(xpart.transpose(0, 2, 1)).reshape(BL, NG, NG, P, S)
            xtp[:, :, 2 * t : 2 * t + 2, :, 2 : 2 + S] = r.transpose(0, 3, 1, 2, 4)
        m = dict(consts)
        m["xt"] = np.ascontiguousarray(xtp).reshape(BL, P, 4 * NG * SP)
        in_maps.append(m)
    return in_maps


_NC_CACHE = {}


def get_nc():
    if "nc" not in _NC_CACHE:
        _NC_CACHE["nc"] = build_nc()
    return _NC_CACHE["nc"]


def unpack_out(arr):
    # [BL, P, NST, D] -> [BL, S, D] (s = st*P + p)
    a = np.asarray(arr, np.float32).reshape(BL, P, NST, D)
    return np.ascontiguousarray(a.transpose(0, 2, 1, 3)).reshape(BL, S, D)


def kernel(**inputs):
    nc = get_nc()
    in_maps = prep_inputs(inputs)
    res = run_bass_kernel_spmd(nc, in_maps, list(range(NCORES)))
    return np.concatenate([unpack_out(r["out"]) for r in res.results], axis=0)


if __name__ == "__main__":
    nc = build_nc()
    print("built ok")
